# revision 9
# baseline (speedup 1.0000x reference)
"""Trainium2 Bass kernel for nn_BareODERegressor (encoder -> 5 RK4(3/8) steps of a
tiny tanh-MLP ODE -> linear readout) over batch B=262144, data-parallel on 8 cores.

Algebraic reformulation: the whole integration runs in "H-space" (the 64-dim
pre-tanh hidden of the circuit MLP). With W21 = dt*(circ_w2 @ circ_w1), each RK4
stage update is a PSUM-accumulated matmul `H += c_i * tanh_i @ W21`; the 6-dim
state y is never materialized. Constants fold into a per-eval ACT bias table and
the readout uses yhat = (H - b1) @ pinv(w1) @ ro_w + ro_b.

Layout: batch chunks of 512 columns; 2 chunks packed along the 128 partitions
("pair"); 2 pairs packed along PSUM free dim ("group" = 2048 batch elems); groups
processed two at a time so the encoder ReLU runs on full 128 partitions.
"""

import os
import sys

import numpy as np

sys.path.insert(0, "/opt/trn_rl_repo")

import concourse.bacc as bacc
import concourse.mybir as mybir
import concourse.tile as tile
from concourse.bass_utils import run_bass_kernel_spmd

F32 = mybir.dt.float32
AF = mybir.ActivationFunctionType

B = 262144
N_CORES = 8
B_CORE = B // N_CORES          # 32768 batch rows per core
NCHUNK = 512                   # batch elems per matmul column block
GROUP = 4 * NCHUNK             # 2048 batch elems (2 pairs of 2 chunks)
N_GP = B_CORE // (2 * GROUP)   # 8 group-pairs per core

DT = 0.2
N_STEPS = 5
C_EVAL = (0.0, 1.0 / 3.0, 2.0 / 3.0, 1.0)
# RK4 (3/8 rule) in-place PSUM delta coefficients. After tanh_e the bank moves
# from H_{e+1} to H_{e+2} via `H += sum_i coef * T_i @ W21`.
COEFS = (1 / 3, -2 / 3, 1.0, 4 / 3, -2.0, -7 / 8, 11 / 8, -5 / 8, 1 / 8)
DELTAS = (
    ((0, 0),),
    ((0, 1), (1, 2)),
    ((0, 3), (1, 4), (2, 2)),
    ((0, 5), (1, 6), (2, 7), (3, 8)),
)

LAST_RESULTS = None
_PROGRAM = None


def _host_params(enc_w1, enc_b1, enc_w2, enc_b2,
                 circ_w1, circ_b1, circ_w2, circ_b2, ro_w, ro_b):
    """Precompute composite matrices / bias tables in float64 on host."""
    e1, eb1, e2, eb2, w1, b1, w2, b2, row, rob = (
        np.asarray(x, np.float64)
        for x in (enc_w1, enc_b1, enc_w2, enc_b2,
                  circ_w1, circ_b1, circ_w2, circ_b2, ro_w, ro_b)
    )
    E21 = e2 @ w1                       # [32, 64]
    W21 = DT * (w2 @ w1)                # [64, 64]
    b21 = DT * (b2 @ w1)                # [64]
    be = eb2 @ w1 + b1                  # [64]
    w1p = w1.T @ np.linalg.inv(w1 @ w1.T)   # right inverse, [64, 6]
    r = w1p @ row                       # [64, 1]
    yconst = float((rob + (be + N_STEPS * b21 - b1) @ r).item())

    LT = np.zeros((128, len(COEFS), 128))
    for k, c in enumerate(COEFS):
        LT[0:64, k, 0:64] = c * W21
        LT[64:128, k, 64:128] = c * W21
    # stored twice (partitions 0-63 and 64-127) so the lhsT slice can share
    # the rhs base partition (PE requires lhsT.base == rhs.base)
    E21bd = np.zeros((128, 128))
    E21bd[0:32, 0:64] = E21
    E21bd[32:64, 64:128] = E21
    E21bd[64:128, :] = E21bd[0:64, :]
    # encoder lhsT: rows (c, f), cols (c, out32) block-diagonal over chunk c
    e1bd = np.zeros((4, 64))
    for c in range(2):
        e1bd[2 * c:2 * c + 2, 32 * c:32 * c + 32] = e1
    rbd = np.zeros((128, 2))
    rbd[0:64, 0] = r[:, 0]
    rbd[64:128, 1] = r[:, 0]
    btab = np.zeros((128, 4 * N_STEPS))
    for s in range(N_STEPS):
        for e in range(4):
            v = be + (s + C_EVAL[e]) * b21
            btab[0:64, 4 * s + e] = v
            btab[64:128, 4 * s + e] = v
    params = {
        "LT": LT,
        "E21bd": E21bd,
        "e1bd": e1bd,                     # [4, 64]
        "rbd": rbd,
        "btab": btab,
        "rbias": np.tile(eb1, 4)[:, None],  # [128, 1]
        "yc": np.full((2, 1), yconst),
    }
    return {k: np.ascontiguousarray(v, np.float32) for k, v in params.items()}


def _build_program():
    nc = bacc.Bacc("TRN2", target_bir_lowering=False, debug=False,
                   num_devices=N_CORES)
    u_ext = nc.dram_tensor("u", [B_CORE, 2], F32, kind="ExternalInput")
    y_ext = nc.dram_tensor("yhat", [B_CORE, 1], F32, kind="ExternalOutput")
    LT_ext = nc.dram_tensor("LT", [128, len(COEFS), 128], F32, kind="ExternalInput")
    E21_ext = nc.dram_tensor("E21bd", [128, 128], F32, kind="ExternalInput")
    e1_ext = nc.dram_tensor("e1bd", [4, 64], F32, kind="ExternalInput")
    rbd_ext = nc.dram_tensor("rbd", [128, 2], F32, kind="ExternalInput")
    btab_ext = nc.dram_tensor("btab", [128, 4 * N_STEPS], F32, kind="ExternalInput")
    rbias_ext = nc.dram_tensor("rbias", [128, 1], F32, kind="ExternalInput")
    yc_ext = nc.dram_tensor("yc", [2, 1], F32, kind="ExternalInput")

    with tile.TileContext(nc) as tc:
        with (
            tc.tile_pool(name="consts", bufs=1) as consts,
            tc.tile_pool(name="upool", bufs=4) as upool,
            tc.tile_pool(name="enchp", bufs=2) as ench_pool,
            tc.tile_pool(name="tpool", bufs=10) as tpool,
            tc.tile_pool(name="hfp", bufs=2) as hf_pool,
            tc.tile_pool(name="ysbp", bufs=4) as ysb_pool,
            tc.tile_pool(name="encp", bufs=1, space="PSUM") as encp_pool,
            tc.tile_pool(name="hp", bufs=2, space="PSUM") as hp_pool,
            tc.tile_pool(name="yp", bufs=2, space="PSUM") as yp_pool,
        ):
            LT_s = consts.tile([128, len(COEFS), 128], F32)
            nc.sync.dma_start(LT_s[:], LT_ext[:, :, :])
            E21_s = consts.tile([128, 128], F32)
            nc.sync.dma_start(E21_s[:], E21_ext[:, :])
            e1_s = consts.tile([4, 64], F32)
            nc.sync.dma_start(e1_s[:], e1_ext[:, :])
            rbd_s = consts.tile([128, 2], F32)
            nc.sync.dma_start(rbd_s[:], rbd_ext[:, :])
            btab_s = consts.tile([128, 4 * N_STEPS], F32)
            nc.sync.dma_start(btab_s[:], btab_ext[:, :])
            rbias_s = consts.tile([128, 1], F32)
            nc.sync.dma_start(rbias_s[:], rbias_ext[:, :])
            yc_s = consts.tile([2, 1], F32)
            nc.sync.dma_start(yc_s[:], yc_ext[:, :])

            for gp in range(N_GP):
                # encoder: two groups share one [128, 1024] PSUM tile so the
                # ReLU runs on all 128 partitions
                enc_ps = encp_pool.tile([128, 2, NCHUNK], F32)
                for gg in (0, 1):
                    base = (2 * gp + gg) * GROUP
                    # u4 partitions ordered (c, f); src rearranged AP sliced
                    # per chunk c since (c f) grouping is not expressible in
                    # a single rearrange
                    u4 = upool.tile([4, 2, NCHUNK], F32)
                    for c in (0, 1):
                        for q in (0, 1):
                            b0 = base + 1024 * q + 512 * c
                            nc.sync.dma_start(
                                u4[2 * c:2 * c + 2, q, :],
                                u_ext[b0:b0 + NCHUNK, :].rearrange(
                                    "n f -> f n"))
                    for q in (0, 1):
                        nc.tensor.matmul(
                            enc_ps[64 * gg:64 * gg + 64, q, :], e1_s[:, :],
                            u4[:, q, :], start=True, stop=True)
                ench = ench_pool.tile([128, 2, NCHUNK], F32)
                nc.scalar.activation(ench[:], enc_ps[:], AF.Relu,
                                     bias=rbias_s[:, 0:1])

                for gg in (0, 1):
                    base = (2 * gp + gg) * GROUP
                    H = hp_pool.tile([128, 2, NCHUNK], F32)
                    for q in (0, 1):
                        nc.tensor.matmul(H[:, q, :],
                                         E21_s[64 * gg:64 * gg + 64, :],
                                         ench[64 * gg:64 * gg + 64, q, :],
                                         start=True, stop=False)
                    for s in range(N_STEPS):
                        T = []
                        for e in range(4):
                            t = tpool.tile([128, 2, NCHUNK], F32, tag="t")
                            nc.scalar.activation(
                                t[:], H[:], AF.Tanh,
                                bias=btab_s[:, 4 * s + e:4 * s + e + 1])
                            T.append(t)
                            last_eval = s == N_STEPS - 1 and e == 3
                            for ci, (ti, slot) in enumerate(DELTAS[e]):
                                last_c = ci == len(DELTAS[e]) - 1
                                for q in (0, 1):
                                    nc.tensor.matmul(
                                        H[:, q, :], LT_s[:, slot, :],
                                        T[ti][:, q, :], start=False,
                                        stop=last_eval and last_c)
                    hf = hf_pool.tile([128, 2, NCHUNK], F32)
                    nc.vector.tensor_copy(hf[:], H[:])
                    for q in (0, 1):
                        ypt = yp_pool.tile([2, NCHUNK], F32)
                        nc.tensor.matmul(ypt[:], rbd_s[:, :], hf[:, q, :],
                                         start=True, stop=True)
                        ys = ysb_pool.tile([2, NCHUNK], F32)
                        nc.vector.tensor_scalar_add(ys[:], ypt[:], yc_s[:, 0:1])
                        nc.sync.dma_start(
                            y_ext[base + 1024 * q:base + 1024 * q + 1024, :]
                            .rearrange("(c n) one -> c (n one)", c=2),
                            ys[:])
    nc.compile()
    return nc


def kernel(u, enc_w1, enc_b1, enc_w2, enc_b2,
           circ_w1, circ_b1, circ_w2, circ_b2, ro_w, ro_b):
    global _PROGRAM, LAST_RESULTS
    params = _host_params(enc_w1, enc_b1, enc_w2, enc_b2,
                          circ_w1, circ_b1, circ_w2, circ_b2, ro_w, ro_b)
    if _PROGRAM is None:
        _PROGRAM = _build_program()
    nc = _PROGRAM

    u = np.ascontiguousarray(np.asarray(u), np.float32)
    in_maps = []
    for c in range(N_CORES):
        m = {"u": u[c * B_CORE:(c + 1) * B_CORE]}
        m.update(params)
        in_maps.append(m)
    trace = os.environ.get("KERNEL_PROFILE", "0") == "1"
    res = run_bass_kernel_spmd(nc, in_maps, list(range(N_CORES)), trace=trace)
    LAST_RESULTS = res
    if trace and res.exec_time_ns is not None:
        print(f"HW exec time: {res.exec_time_ns} ns "
              f"(mean {res.mean_exec_time_ns} ns, "
              f"max core {res.max_exec_time_core_id})")
    yhat = np.concatenate([res.results[c]["yhat"] for c in range(N_CORES)],
                          axis=0).astype(np.float32)
    return yhat, 4 * N_STEPS


# revision 10
# speedup vs baseline: 1.7426x; 1.7426x over previous
"""Trainium2 Bass kernel for nn_BareODERegressor (encoder -> 5 RK4(3/8) steps of a
tiny tanh-MLP ODE -> linear readout) over batch B=262144, data-parallel on 8 cores.

Algebraic reformulation: the whole integration runs in "H-space" (the 64-dim
pre-tanh hidden of the circuit MLP). With W21 = dt*(circ_w2 @ circ_w1), each RK4
stage update is a PSUM-accumulated matmul `H += c_i * tanh_i @ W21`; the 6-dim
state y is never materialized. Constants fold into a per-eval ACT bias table and
the readout uses yhat = (H - b1) @ pinv(w1) @ ro_w + ro_b.

Layout: batch chunks of 512 columns; 2 chunks packed along the 128 partitions
("pair"); 2 pairs packed along PSUM free dim ("group" = 2048 batch elems); groups
processed two at a time so the encoder ReLU runs on full 128 partitions.
"""

import os
import sys

import numpy as np

sys.path.insert(0, "/opt/trn_rl_repo")

import concourse.bacc as bacc
import concourse.mybir as mybir
import concourse.tile as tile
from concourse.bass_utils import run_bass_kernel_spmd

F32 = mybir.dt.float32
F16 = mybir.dt.float16
AF = mybir.ActivationFunctionType

B = 262144
N_CORES = 8
B_CORE = B // N_CORES          # 32768 batch rows per core
NCHUNK = 512                   # batch elems per matmul column block
GROUP = 4 * NCHUNK             # 2048 batch elems (2 pairs of 2 chunks)
N_GP = B_CORE // (2 * GROUP)   # 8 group-pairs per core

DT = 0.2
N_STEPS = 5
C_EVAL = (0.0, 1.0 / 3.0, 2.0 / 3.0, 1.0)
# RK4 (3/8 rule) in-place PSUM delta coefficients. After tanh_e the bank moves
# from H_{e+1} to H_{e+2} via `H += sum_i coef * T_i @ W21`.
COEFS = (1 / 3, -2 / 3, 1.0, 4 / 3, -2.0, -7 / 8, 11 / 8, -5 / 8, 1 / 8)
DELTAS = (
    ((0, 0),),
    ((0, 1), (1, 2)),
    ((0, 3), (1, 4), (2, 2)),
    ((0, 5), (1, 6), (2, 7), (3, 8)),
)

LAST_RESULTS = None
_PROGRAM = None


def _host_params(enc_w1, enc_b1, enc_w2, enc_b2,
                 circ_w1, circ_b1, circ_w2, circ_b2, ro_w, ro_b):
    """Precompute composite matrices / bias tables in float64 on host."""
    e1, eb1, e2, eb2, w1, b1, w2, b2, row, rob = (
        np.asarray(x, np.float64)
        for x in (enc_w1, enc_b1, enc_w2, enc_b2,
                  circ_w1, circ_b1, circ_w2, circ_b2, ro_w, ro_b)
    )
    E21 = e2 @ w1                       # [32, 64]
    W21 = DT * (w2 @ w1)                # [64, 64]
    b21 = DT * (b2 @ w1)                # [64]
    be = eb2 @ w1 + b1                  # [64]
    w1p = w1.T @ np.linalg.inv(w1 @ w1.T)   # right inverse, [64, 6]
    r = w1p @ row                       # [64, 1]
    yconst = float((rob + (be + N_STEPS * b21 - b1) @ r).item())

    LT = np.zeros((128, len(COEFS), 128))
    for k, c in enumerate(COEFS):
        LT[0:64, k, 0:64] = c * W21
        LT[64:128, k, 64:128] = c * W21
    # stored twice (partitions 0-63 and 64-127) so the lhsT slice can share
    # the rhs base partition (PE requires lhsT.base == rhs.base)
    E21bd = np.zeros((128, 128))
    E21bd[0:32, 0:64] = E21
    E21bd[32:64, 64:128] = E21
    E21bd[64:128, :] = E21bd[0:64, :]
    # encoder lhsT: rows (c, f), cols (c, out32) block-diagonal over chunk c
    e1bd = np.zeros((4, 64))
    for c in range(2):
        e1bd[2 * c:2 * c + 2, 32 * c:32 * c + 32] = e1
    rbd = np.zeros((128, 2))
    rbd[0:64, 0] = r[:, 0]
    rbd[64:128, 1] = r[:, 0]
    btab = np.zeros((128, 4 * N_STEPS))
    for s in range(N_STEPS):
        for e in range(4):
            v = be + (s + C_EVAL[e]) * b21
            btab[0:64, 4 * s + e] = v
            btab[64:128, 4 * s + e] = v
    params = {
        "LT": LT,  # cast to f16 below
        "E21bd": E21bd,
        "e1bd": e1bd,                     # [4, 64]
        "rbd": rbd,
        "btab": btab,
        "rbias": np.tile(eb1, 4)[:, None],  # [128, 1]
        "yc": np.full((2, 1), yconst),
    }
    out = {k: np.ascontiguousarray(v, np.float32) for k, v in params.items()}
    out["LT"] = np.ascontiguousarray(LT, np.float16)
    return out


def _build_program():
    nc = bacc.Bacc("TRN2", target_bir_lowering=False, debug=False,
                   num_devices=N_CORES)
    u_ext = nc.dram_tensor("u", [B_CORE, 2], F32, kind="ExternalInput")
    y_ext = nc.dram_tensor("yhat", [B_CORE, 1], F32, kind="ExternalOutput")
    LT_ext = nc.dram_tensor("LT", [128, len(COEFS), 128], F16, kind="ExternalInput")
    E21_ext = nc.dram_tensor("E21bd", [128, 128], F32, kind="ExternalInput")
    e1_ext = nc.dram_tensor("e1bd", [4, 64], F32, kind="ExternalInput")
    rbd_ext = nc.dram_tensor("rbd", [128, 2], F32, kind="ExternalInput")
    btab_ext = nc.dram_tensor("btab", [128, 4 * N_STEPS], F32, kind="ExternalInput")
    rbias_ext = nc.dram_tensor("rbias", [128, 1], F32, kind="ExternalInput")
    yc_ext = nc.dram_tensor("yc", [2, 1], F32, kind="ExternalInput")

    with tile.TileContext(nc) as tc:
        with (
            tc.tile_pool(name="consts", bufs=1) as consts,
            tc.tile_pool(name="upool", bufs=4) as upool,
            tc.tile_pool(name="enchp", bufs=2) as ench_pool,
            tc.tile_pool(name="tpool", bufs=10) as tpool,
            tc.tile_pool(name="hfp", bufs=2) as hf_pool,
            tc.tile_pool(name="ysbp", bufs=4) as ysb_pool,
            tc.tile_pool(name="encp", bufs=1, space="PSUM") as encp_pool,
            tc.tile_pool(name="hp", bufs=2, space="PSUM") as hp_pool,
            tc.tile_pool(name="yp", bufs=2, space="PSUM") as yp_pool,
        ):
            LT_s = consts.tile([128, len(COEFS), 128], F16)
            nc.sync.dma_start(LT_s[:], LT_ext[:, :, :])
            E21_s = consts.tile([128, 128], F32)
            nc.sync.dma_start(E21_s[:], E21_ext[:, :])
            e1_s = consts.tile([4, 64], F32)
            nc.sync.dma_start(e1_s[:], e1_ext[:, :])
            rbd_s = consts.tile([128, 2], F32)
            nc.sync.dma_start(rbd_s[:], rbd_ext[:, :])
            btab_s = consts.tile([128, 4 * N_STEPS], F32)
            nc.sync.dma_start(btab_s[:], btab_ext[:, :])
            rbias_s = consts.tile([128, 1], F32)
            nc.sync.dma_start(rbias_s[:], rbias_ext[:, :])
            yc_s = consts.tile([2, 1], F32)
            nc.sync.dma_start(yc_s[:], yc_ext[:, :])

            for gp in range(N_GP):
                # encoder: two groups share one [128, 1024] PSUM tile so the
                # ReLU runs on all 128 partitions
                enc_ps = encp_pool.tile([128, 2, NCHUNK], F32)
                for gg in (0, 1):
                    base = (2 * gp + gg) * GROUP
                    # u4 partitions ordered (c, f); src rearranged AP sliced
                    # per chunk c since (c f) grouping is not expressible in
                    # a single rearrange
                    u4 = upool.tile([4, 2, NCHUNK], F32)
                    for c in (0, 1):
                        for q in (0, 1):
                            b0 = base + 1024 * q + 512 * c
                            nc.sync.dma_start(
                                u4[2 * c:2 * c + 2, q, :],
                                u_ext[b0:b0 + NCHUNK, :].rearrange(
                                    "n f -> f n"))
                    for q in (0, 1):
                        nc.tensor.matmul(
                            enc_ps[64 * gg:64 * gg + 64, q, :], e1_s[:, :],
                            u4[:, q, :], start=True, stop=True)
                ench = ench_pool.tile([128, 2, NCHUNK], F32)
                nc.scalar.activation(ench[:], enc_ps[:], AF.Relu,
                                     bias=rbias_s[:, 0:1])

                for gg in (0, 1):
                    base = (2 * gp + gg) * GROUP
                    H = hp_pool.tile([128, 2, NCHUNK], F32)
                    for q in (0, 1):
                        nc.tensor.matmul(H[:, q, :],
                                         E21_s[64 * gg:64 * gg + 64, :],
                                         ench[64 * gg:64 * gg + 64, q, :],
                                         start=True, stop=False)
                    for s in range(N_STEPS):
                        T = []
                        for e in range(4):
                            t = tpool.tile([128, 2, NCHUNK], F16, tag="t")
                            nc.scalar.activation(
                                t[:], H[:], AF.Tanh,
                                bias=btab_s[:, 4 * s + e:4 * s + e + 1])
                            T.append(t)
                            last_eval = s == N_STEPS - 1 and e == 3
                            for ci, (ti, slot) in enumerate(DELTAS[e]):
                                last_c = ci == len(DELTAS[e]) - 1
                                for q in (0, 1):
                                    nc.tensor.matmul(
                                        H[:, q, :], LT_s[:, slot, :],
                                        T[ti][:, q, :], start=False,
                                        stop=last_eval and last_c)
                    hf = hf_pool.tile([128, 2, NCHUNK], F32)
                    nc.vector.tensor_copy(hf[:], H[:])
                    for q in (0, 1):
                        ypt = yp_pool.tile([2, NCHUNK], F32)
                        nc.tensor.matmul(ypt[:], rbd_s[:, :], hf[:, q, :],
                                         start=True, stop=True)
                        ys = ysb_pool.tile([2, NCHUNK], F32)
                        nc.vector.tensor_scalar_add(ys[:], ypt[:], yc_s[:, 0:1])
                        nc.sync.dma_start(
                            y_ext[base + 1024 * q:base + 1024 * q + 1024, :]
                            .rearrange("(c n) one -> c (n one)", c=2),
                            ys[:])
    nc.compile()
    return nc


def kernel(u, enc_w1, enc_b1, enc_w2, enc_b2,
           circ_w1, circ_b1, circ_w2, circ_b2, ro_w, ro_b):
    global _PROGRAM, LAST_RESULTS
    params = _host_params(enc_w1, enc_b1, enc_w2, enc_b2,
                          circ_w1, circ_b1, circ_w2, circ_b2, ro_w, ro_b)
    if _PROGRAM is None:
        _PROGRAM = _build_program()
    nc = _PROGRAM

    u = np.ascontiguousarray(np.asarray(u), np.float32)
    in_maps = []
    for c in range(N_CORES):
        m = {"u": u[c * B_CORE:(c + 1) * B_CORE]}
        m.update(params)
        in_maps.append(m)
    trace = os.environ.get("KERNEL_PROFILE", "0") == "1"
    res = run_bass_kernel_spmd(nc, in_maps, list(range(N_CORES)), trace=trace)
    LAST_RESULTS = res
    if trace and res.exec_time_ns is not None:
        print(f"HW exec time: {res.exec_time_ns} ns "
              f"(mean {res.mean_exec_time_ns} ns, "
              f"max core {res.max_exec_time_core_id})")
    yhat = np.concatenate([res.results[c]["yhat"] for c in range(N_CORES)],
                          axis=0).astype(np.float32)
    return yhat, 4 * N_STEPS


# revision 11
# speedup vs baseline: 1.7779x; 1.0202x over previous
"""Trainium2 Bass kernel for nn_BareODERegressor (encoder -> 5 RK4(3/8) steps of a
tiny tanh-MLP ODE -> linear readout) over batch B=262144, data-parallel on 8 cores.

Algebraic reformulation: the whole integration runs in "H-space" (the 64-dim
pre-tanh hidden of the circuit MLP). With W21 = dt*(circ_w2 @ circ_w1), each RK4
stage update is a PSUM-accumulated matmul `H += c_i * tanh_i @ W21`; the 6-dim
state y is never materialized. Constants fold into a per-eval ACT bias table and
the readout uses yhat = (H - b1) @ pinv(w1) @ ro_w + ro_b.

Layout: batch chunks of 512 columns; 2 chunks packed along the 128 partitions;
group = 2 chunk-pairs = one [128, 2, 512] PSUM tile (2 banks). 4 groups in
flight (8 banks); the encoder and the readout phases reuse the group's own
banks, so PSUM is exactly full. Matmul operands are fp16 (PSUM accumulation
stays fp32) except the readout matmul, which stays fp32.
"""

import os
import sys

import numpy as np

sys.path.insert(0, "/opt/trn_rl_repo")

import concourse.bacc as bacc
import concourse.mybir as mybir
import concourse.tile as tile
from concourse.bass_utils import run_bass_kernel_spmd

F32 = mybir.dt.float32
F16 = mybir.dt.float16
AF = mybir.ActivationFunctionType

B = 262144
N_CORES = 8
B_CORE = B // N_CORES          # 32768 batch rows per core
NCHUNK = 512                   # batch elems per matmul column block
GROUP = 4 * NCHUNK             # 2048 batch elems (2 pairs of 2 chunks)
N_G = B_CORE // GROUP          # 16 groups per core

DT = 0.2
N_STEPS = 5
C_EVAL = (0.0, 1.0 / 3.0, 2.0 / 3.0, 1.0)
# RK4 (3/8 rule) in-place PSUM delta coefficients. After tanh_e the bank moves
# from H_{e+1} to H_{e+2} via `H += sum_i coef * T_i @ W21`.
COEFS = (1 / 3, -2 / 3, 1.0, 4 / 3, -2.0, -7 / 8, 11 / 8, -5 / 8, 1 / 8)
DELTAS = (
    ((0, 0),),
    ((0, 1), (1, 2)),
    ((0, 3), (1, 4), (2, 2)),
    ((0, 5), (1, 6), (2, 7), (3, 8)),
)

LAST_RESULTS = None
_PROGRAM = None


def _host_params(enc_w1, enc_b1, enc_w2, enc_b2,
                 circ_w1, circ_b1, circ_w2, circ_b2, ro_w, ro_b):
    """Precompute composite matrices / bias tables in float64 on host."""
    e1, eb1, e2, eb2, w1, b1, w2, b2, row, rob = (
        np.asarray(x, np.float64)
        for x in (enc_w1, enc_b1, enc_w2, enc_b2,
                  circ_w1, circ_b1, circ_w2, circ_b2, ro_w, ro_b)
    )
    E21 = e2 @ w1                       # [32, 64]
    W21 = DT * (w2 @ w1)                # [64, 64]
    b21 = DT * (b2 @ w1)                # [64]
    be = eb2 @ w1 + b1                  # [64]
    w1p = w1.T @ np.linalg.inv(w1 @ w1.T)   # right inverse, [64, 6]
    r = w1p @ row                       # [64, 1]
    yconst = float((rob + (be + N_STEPS * b21 - b1) @ r).item())

    LT = np.zeros((128, len(COEFS), 128))
    for k, c in enumerate(COEFS):
        LT[0:64, k, 0:64] = c * W21
        LT[64:128, k, 64:128] = c * W21
    # H1 lhsT: rows (c, enc32), cols (c, h64), block-diagonal over chunk c
    E21bd = np.zeros((64, 128))
    E21bd[0:32, 0:64] = E21
    E21bd[32:64, 64:128] = E21
    # encoder lhsT: rows (c, f), cols (c, out32), block-diagonal over chunk c
    e1bd = np.zeros((4, 64))
    for c in range(2):
        e1bd[2 * c:2 * c + 2, 32 * c:32 * c + 32] = e1
    rbd = np.zeros((128, 2))
    rbd[0:64, 0] = r[:, 0]
    rbd[64:128, 1] = r[:, 0]
    btab = np.zeros((128, 4 * N_STEPS))
    for s in range(N_STEPS):
        for e in range(4):
            v = be + (s + C_EVAL[e]) * b21
            btab[0:64, 4 * s + e] = v
            btab[64:128, 4 * s + e] = v
    fp32 = {
        "rbd": rbd,
        "btab": btab,
        "rbias": np.tile(eb1, 2)[:, None],  # [64, 1] for the 64-lane ReLU
        "yc": np.full((2, 1), yconst),
    }
    fp16 = {"LT": LT, "E21bd": E21bd, "e1bd": e1bd}
    out = {k: np.ascontiguousarray(v, np.float32) for k, v in fp32.items()}
    out.update({k: np.ascontiguousarray(v, np.float16)
                for k, v in fp16.items()})
    return out


def _build_program():
    nc = bacc.Bacc("TRN2", target_bir_lowering=False, debug=False,
                   num_devices=N_CORES)
    u_ext = nc.dram_tensor("u", [B_CORE, 2], F16, kind="ExternalInput")
    y_ext = nc.dram_tensor("yhat", [B_CORE, 1], F32, kind="ExternalOutput")
    LT_ext = nc.dram_tensor("LT", [128, len(COEFS), 128], F16,
                            kind="ExternalInput")
    E21_ext = nc.dram_tensor("E21bd", [64, 128], F16, kind="ExternalInput")
    e1_ext = nc.dram_tensor("e1bd", [4, 64], F16, kind="ExternalInput")
    rbd_ext = nc.dram_tensor("rbd", [128, 2], F32, kind="ExternalInput")
    btab_ext = nc.dram_tensor("btab", [128, 4 * N_STEPS], F32,
                              kind="ExternalInput")
    rbias_ext = nc.dram_tensor("rbias", [64, 1], F32, kind="ExternalInput")
    yc_ext = nc.dram_tensor("yc", [2, 1], F32, kind="ExternalInput")

    with tile.TileContext(nc) as tc:
        with (
            tc.tile_pool(name="consts", bufs=1) as consts,
            tc.tile_pool(name="upool", bufs=6) as upool,
            tc.tile_pool(name="enchp", bufs=4) as ench_pool,
            tc.tile_pool(name="tpool", bufs=16) as tpool,
            tc.tile_pool(name="hfp", bufs=3) as hf_pool,
            tc.tile_pool(name="ysbp", bufs=4) as ysb_pool,
            tc.tile_pool(name="hp", bufs=4, space="PSUM") as hp_pool,
        ):
            LT_s = consts.tile([128, len(COEFS), 128], F16)
            nc.sync.dma_start(LT_s[:], LT_ext[:, :, :])
            E21_s = consts.tile([64, 128], F16)
            nc.sync.dma_start(E21_s[:], E21_ext[:, :])
            e1_s = consts.tile([4, 64], F16)
            nc.sync.dma_start(e1_s[:], e1_ext[:, :])
            rbd_s = consts.tile([128, 2], F32)
            nc.sync.dma_start(rbd_s[:], rbd_ext[:, :])
            btab_s = consts.tile([128, 4 * N_STEPS], F32)
            nc.sync.dma_start(btab_s[:], btab_ext[:, :])
            rbias_s = consts.tile([64, 1], F32)
            nc.sync.dma_start(rbias_s[:], rbias_ext[:, :])
            yc_s = consts.tile([2, 1], F32)
            nc.sync.dma_start(yc_s[:], yc_ext[:, :])

            for g in range(N_G):
                base = g * GROUP
                H = hp_pool.tile([128, 2, NCHUNK], F32)
                # encoder phase reuses the H banks: partitions (c*32+f) x (q)
                u4 = upool.tile([4, 2, NCHUNK], F16)
                for c in (0, 1):
                    for q in (0, 1):
                        b0 = base + 1024 * q + 512 * c
                        nc.sync.dma_start(
                            u4[2 * c:2 * c + 2, q, :],
                            u_ext[b0:b0 + NCHUNK, :].rearrange("n f -> f n"))
                for q in (0, 1):
                    nc.tensor.matmul(H[0:64, q, :], e1_s[:, :], u4[:, q, :],
                                     start=True, stop=True)
                ench = ench_pool.tile([64, 2, NCHUNK], F16)
                nc.scalar.activation(ench[:], H[0:64, :, :], AF.Relu,
                                     bias=rbias_s[:, 0:1])
                # H1 init overwrites the encoder residue (start=True)
                for q in (0, 1):
                    nc.tensor.matmul(H[:, q, :], E21_s[:, :], ench[:, q, :],
                                     start=True, stop=False)
                for s in range(N_STEPS):
                    T = []
                    for e in range(4):
                        t = tpool.tile([128, 2, NCHUNK], F16, tag="t")
                        nc.scalar.activation(
                            t[:], H[:], AF.Tanh,
                            bias=btab_s[:, 4 * s + e:4 * s + e + 1])
                        T.append(t)
                        last_eval = s == N_STEPS - 1 and e == 3
                        for ci, (ti, slot) in enumerate(DELTAS[e]):
                            last_c = ci == len(DELTAS[e]) - 1
                            for q in (0, 1):
                                nc.tensor.matmul(
                                    H[:, q, :], LT_s[:, slot, :],
                                    T[ti][:, q, :], start=False,
                                    stop=last_eval and last_c)
                hf = hf_pool.tile([128, 2, NCHUNK], F32)
                nc.vector.tensor_copy(hf[:], H[:])
                # readout (fp32) writes into partitions 0-1 of the freed banks
                for q in (0, 1):
                    nc.tensor.matmul(H[0:2, q, :], rbd_s[:, :], hf[:, q, :],
                                     start=True, stop=True)
                ys = ysb_pool.tile([2, 2, NCHUNK], F32)
                nc.vector.tensor_scalar_add(ys[:], H[0:2, :, :], yc_s[:, 0:1])
                nc.sync.dma_start(
                    y_ext[base:base + GROUP, :].rearrange(
                        "(q c n) one -> c q (n one)", q=2, c=2),
                    ys[:])
    nc.compile()
    return nc


def kernel(u, enc_w1, enc_b1, enc_w2, enc_b2,
           circ_w1, circ_b1, circ_w2, circ_b2, ro_w, ro_b):
    global _PROGRAM, LAST_RESULTS
    params = _host_params(enc_w1, enc_b1, enc_w2, enc_b2,
                          circ_w1, circ_b1, circ_w2, circ_b2, ro_w, ro_b)
    if _PROGRAM is None:
        _PROGRAM = _build_program()
    nc = _PROGRAM

    u = np.ascontiguousarray(np.asarray(u), np.float16)
    in_maps = []
    for c in range(N_CORES):
        m = {"u": u[c * B_CORE:(c + 1) * B_CORE]}
        m.update(params)
        in_maps.append(m)
    trace = os.environ.get("KERNEL_PROFILE", "0") == "1"
    res = run_bass_kernel_spmd(nc, in_maps, list(range(N_CORES)), trace=trace)
    LAST_RESULTS = res
    if trace and res.exec_time_ns is not None:
        print(f"HW exec time: {res.exec_time_ns} ns "
              f"(mean {res.mean_exec_time_ns} ns, "
              f"max core {res.max_exec_time_core_id})")
    yhat = np.concatenate([res.results[c]["yhat"] for c in range(N_CORES)],
                          axis=0).astype(np.float32)
    return yhat, 4 * N_STEPS


# revision 13
# speedup vs baseline: 4.1541x; 2.3366x over previous
"""Trainium2 Bass kernel for nn_BareODERegressor (encoder -> 5 RK4(3/8) steps of a
tiny tanh-MLP ODE -> linear readout) over batch B=262144, data-parallel on 8 cores.

Algebraic reformulation: the whole integration runs in "H-space" (the 64-dim
pre-tanh hidden of the circuit MLP). With W21 = dt*(circ_w2 @ circ_w1), each RK4
stage update is a PSUM-accumulated matmul `H += c_i * tanh_i @ W21`; the 6-dim
state y is never materialized. Constants fold into a per-eval ACT bias table and
the readout uses yhat = (H - b1) @ pinv(w1) @ ro_w + ro_b.

Layout: batch chunks of 512 columns; 2 chunks packed along the 128 partitions;
group = 2 chunk-pairs = one [128, 2, 512] PSUM tile (2 banks). 4 groups in
flight (8 banks); the encoder and the readout phases reuse the group's own
banks, so PSUM is exactly full. Matmul operands are fp16 (PSUM accumulation
stays fp32) except the readout matmul, which stays fp32.
"""

import os
import sys

import numpy as np

sys.path.insert(0, "/opt/trn_rl_repo")

import concourse.bacc as bacc
import concourse.mybir as mybir
import concourse.tile as tile
from concourse.bass_utils import run_bass_kernel_spmd

F32 = mybir.dt.float32
F16 = mybir.dt.float16
AF = mybir.ActivationFunctionType

B = 262144
N_CORES = 8
B_CORE = B // N_CORES          # 32768 batch rows per core
NCHUNK = 512                   # batch elems per matmul column block
GROUP = 4 * NCHUNK             # 2048 batch elems (2 pairs of 2 chunks)
N_G = B_CORE // GROUP          # 16 groups per core

DT = 0.2
N_STEPS = 5
C_EVAL = (0.0, 1.0 / 3.0, 2.0 / 3.0, 1.0)
# RK4 (3/8 rule) in-place PSUM delta coefficients. After tanh_e the bank moves
# from H_{e+1} to H_{e+2} via `H += sum_i coef * T_i @ W21`.
COEFS = (1 / 3, -2 / 3, 1.0, 4 / 3, -2.0, -7 / 8, 11 / 8, -5 / 8, 1 / 8)
DELTAS = (
    ((0, 0),),
    ((0, 1), (1, 2)),
    ((0, 3), (1, 4), (2, 2)),
    ((0, 5), (1, 6), (2, 7), (3, 8)),
)

LAST_RESULTS = None
_PROGRAM = None


def _host_params(enc_w1, enc_b1, enc_w2, enc_b2,
                 circ_w1, circ_b1, circ_w2, circ_b2, ro_w, ro_b):
    """Precompute composite matrices / bias tables in float64 on host."""
    e1, eb1, e2, eb2, w1, b1, w2, b2, row, rob = (
        np.asarray(x, np.float64)
        for x in (enc_w1, enc_b1, enc_w2, enc_b2,
                  circ_w1, circ_b1, circ_w2, circ_b2, ro_w, ro_b)
    )
    E21 = e2 @ w1                       # [32, 64]
    W21 = DT * (w2 @ w1)                # [64, 64]
    b21 = DT * (b2 @ w1)                # [64]
    be = eb2 @ w1 + b1                  # [64]
    w1p = w1.T @ np.linalg.inv(w1 @ w1.T)   # right inverse, [64, 6]
    r = w1p @ row                       # [64, 1]
    yconst = float((rob + (be + N_STEPS * b21 - b1) @ r).item())

    LT = np.zeros((128, len(COEFS), 128))
    for k, c in enumerate(COEFS):
        LT[0:64, k, 0:64] = c * W21
        LT[64:128, k, 64:128] = c * W21
    # H1 lhsT: rows (c, enc32), cols (c, h64), block-diagonal over chunk c
    E21bd = np.zeros((64, 128))
    E21bd[0:32, 0:64] = E21
    E21bd[32:64, 64:128] = E21
    # encoder lhsT: rows (c, f), cols (c, out32), block-diagonal over chunk c
    e1bd = np.zeros((4, 64))
    for c in range(2):
        e1bd[2 * c:2 * c + 2, 32 * c:32 * c + 32] = e1
    rbd = np.zeros((128, 2))
    rbd[0:64, 0] = r[:, 0]
    rbd[64:128, 1] = r[:, 0]
    btab = np.zeros((128, 4 * N_STEPS))
    for s in range(N_STEPS):
        for e in range(4):
            v = be + (s + C_EVAL[e]) * b21
            btab[0:64, 4 * s + e] = v
            btab[64:128, 4 * s + e] = v
    fp32 = {
        "btab": btab,
        "rbias": np.tile(eb1, 2)[:, None],  # [64, 1] for the 64-lane ReLU
        "yc": np.full((2, 1), yconst),
    }
    fp16 = {"LT": LT, "E21bd": E21bd, "e1bd": e1bd, "rbd": rbd}
    out = {k: np.ascontiguousarray(v, np.float32) for k, v in fp32.items()}
    out.update({k: np.ascontiguousarray(v, np.float16)
                for k, v in fp16.items()})
    return out


def _build_program():
    nc = bacc.Bacc("TRN2", target_bir_lowering=False, debug=False,
                   num_devices=N_CORES)
    u_ext = nc.dram_tensor("u", [2, B_CORE], F16, kind="ExternalInput")
    y_ext = nc.dram_tensor("yhat", [B_CORE, 1], F32, kind="ExternalOutput")
    LT_ext = nc.dram_tensor("LT", [128, len(COEFS), 128], F16,
                            kind="ExternalInput")
    E21_ext = nc.dram_tensor("E21bd", [64, 128], F16, kind="ExternalInput")
    e1_ext = nc.dram_tensor("e1bd", [4, 64], F16, kind="ExternalInput")
    rbd_ext = nc.dram_tensor("rbd", [128, 2], F16, kind="ExternalInput")
    btab_ext = nc.dram_tensor("btab", [128, 4 * N_STEPS], F32,
                              kind="ExternalInput")
    rbias_ext = nc.dram_tensor("rbias", [64, 1], F32, kind="ExternalInput")
    yc_ext = nc.dram_tensor("yc", [2, 1], F32, kind="ExternalInput")

    with tile.TileContext(nc) as tc:
        with (
            tc.tile_pool(name="consts", bufs=1) as consts,
            tc.tile_pool(name="upool", bufs=6) as upool,
            tc.tile_pool(name="enchp", bufs=4) as ench_pool,
            tc.tile_pool(name="tpool", bufs=16) as tpool,
            tc.tile_pool(name="hfp", bufs=3) as hf_pool,
            tc.tile_pool(name="ysbp", bufs=4) as ysb_pool,
            tc.tile_pool(name="hp", bufs=4, space="PSUM") as hp_pool,
        ):
            LT_s = consts.tile([128, len(COEFS), 128], F16)
            nc.sync.dma_start(LT_s[:], LT_ext[:, :, :])
            E21_s = consts.tile([64, 128], F16)
            nc.sync.dma_start(E21_s[:], E21_ext[:, :])
            e1_s = consts.tile([4, 64], F16)
            nc.sync.dma_start(e1_s[:], e1_ext[:, :])
            rbd_s = consts.tile([128, 2], F16)
            nc.sync.dma_start(rbd_s[:], rbd_ext[:, :])
            btab_s = consts.tile([128, 4 * N_STEPS], F32)
            nc.sync.dma_start(btab_s[:], btab_ext[:, :])
            rbias_s = consts.tile([64, 1], F32)
            nc.sync.dma_start(rbias_s[:], rbias_ext[:, :])
            yc_s = consts.tile([2, 1], F32)
            nc.sync.dma_start(yc_s[:], yc_ext[:, :])

            NB = 4                       # groups per batch = psum bufs
            for gb in range(N_G // NB):
                Hs, u4s, enchs = [], [], []
                for j in range(NB):
                    base = (NB * gb + j) * GROUP
                    H = hp_pool.tile([128, 2, NCHUNK], F32, tag="h")
                    # u arrives pre-transposed [2, B]; contiguous loads
                    u4 = upool.tile([4, 2, NCHUNK], F16, tag="u")
                    usrc = u_ext[:, base:base + GROUP].rearrange(
                        "f (q c2 n) -> c2 f q n", q=2, c2=2)
                    for c in (0, 1):
                        nc.sync.dma_start(u4[2 * c:2 * c + 2, :, :],
                                          usrc[c, :, :, :])
                    for q in (0, 1):
                        nc.tensor.matmul(H[0:64, q, :], e1_s[:, :],
                                         u4[:, q, :], start=True, stop=True)
                    Hs.append(H)
                    u4s.append(u4)
                for j in range(NB):
                    ench = ench_pool.tile([64, 2, NCHUNK], F16, tag="ench")
                    nc.scalar.activation(ench[:], Hs[j][0:64, :, :], AF.Relu,
                                         bias=rbias_s[:, 0:1])
                    enchs.append(ench)
                for j in range(NB):
                    for q in (0, 1):
                        nc.tensor.matmul(Hs[j][:, q, :], E21_s[:, :],
                                         enchs[j][:, q, :],
                                         start=True, stop=False)
                for s in range(N_STEPS):
                    T = [[] for _ in range(NB)]
                    for e in range(4):
                        for j in range(NB):
                            t = tpool.tile([128, 2, NCHUNK], F16, tag="t")
                            nc.scalar.activation(
                                t[:], Hs[j][:], AF.Tanh,
                                bias=btab_s[:, 4 * s + e:4 * s + e + 1])
                            T[j].append(t)
                        last_eval = s == N_STEPS - 1 and e == 3
                        # contribution-major: consecutive matmuls share lhsT
                        for ci, (ti, slot) in enumerate(DELTAS[e]):
                            last_c = ci == len(DELTAS[e]) - 1
                            for j in range(NB):
                                for q in (0, 1):
                                    nc.tensor.matmul(
                                        Hs[j][:, q, :], LT_s[:, slot, :],
                                        T[j][ti][:, q, :], start=False,
                                        stop=last_eval and last_c)
                for j in range(NB):
                    base = (NB * gb + j) * GROUP
                    H = Hs[j]
                    hf = hf_pool.tile([128, 2, NCHUNK], F16, tag="hf")
                    nc.vector.tensor_copy(hf[:], H[:])
                    for q in (0, 1):
                        nc.tensor.matmul(H[0:2, q, :], rbd_s[:, :],
                                         hf[:, q, :], start=True, stop=True)
                    ys = ysb_pool.tile([2, 2, NCHUNK], F32, tag="ys")
                    nc.vector.tensor_scalar_add(ys[:], H[0:2, :, :],
                                                yc_s[:, 0:1])
                    nc.sync.dma_start(
                        y_ext[base:base + GROUP, :].rearrange(
                            "(q c n) one -> c q (n one)", q=2, c=2),
                        ys[:])
    nc.compile()
    return nc


def kernel(u, enc_w1, enc_b1, enc_w2, enc_b2,
           circ_w1, circ_b1, circ_w2, circ_b2, ro_w, ro_b):
    global _PROGRAM, LAST_RESULTS
    params = _host_params(enc_w1, enc_b1, enc_w2, enc_b2,
                          circ_w1, circ_b1, circ_w2, circ_b2, ro_w, ro_b)
    if _PROGRAM is None:
        _PROGRAM = _build_program()
    nc = _PROGRAM

    u = np.ascontiguousarray(np.asarray(u).T, np.float16)  # [2, B]
    in_maps = []
    for c in range(N_CORES):
        m = {"u": u[:, c * B_CORE:(c + 1) * B_CORE]}
        m.update(params)
        in_maps.append(m)
    trace = os.environ.get("KERNEL_PROFILE", "0") == "1"
    res = run_bass_kernel_spmd(nc, in_maps, list(range(N_CORES)), trace=trace)
    LAST_RESULTS = res
    if trace and res.exec_time_ns is not None:
        print(f"HW exec time: {res.exec_time_ns} ns "
              f"(mean {res.mean_exec_time_ns} ns, "
              f"max core {res.max_exec_time_core_id})")
    yhat = np.concatenate([res.results[c]["yhat"] for c in range(N_CORES)],
                          axis=0).astype(np.float32)
    return yhat, 4 * N_STEPS
